# revision 1
# baseline (speedup 1.0000x reference)
"""GAT layer (nn_GAT_40037685133531) as a Trainium2 Bass kernel on 8 NeuronCores.

Strategy (graph/data parallel, no collectives):
  - Destination nodes sharded 8 ways (6250 per core); edges sorted by dst and
    assigned to the core owning their destination.
  - Phase 0 (replicated on every core): h_ext = x @ [W | W@A_src | W@A_dst]
    -> htab [N, 512B rows]: h in bf16 (256B) + alpha_s in f32 (16B) + pad;
    alpha_d separately in a compact f32 table ad_tab [N, 4].
  - Phase 1 (per 128-destination-node tile): dma_gather fetches the 512B rows
    of all the tile's edges by src index (two gathers: int16 indices force a
    lo/hi table split at row 32768). alpha_d for the tile's 128 nodes comes
    from one [P,1]-indexed indirect DMA on ad_tab and is broadcast to edges
    with a PE matmul against S_T. Attention weights ex = exp(leakyrelu(
    alpha_s+alpha_d)) are computed per edge, messages msg = [ex*h | ex], and
    the one-hot scatter matrix S (one is_equal over an iota; S_T by DVE
    transpose) accumulates psum += S.T @ msg on the TensorEngine, giving both
    weighted feature sums and softmax denominators.
  - Softmax normalization happens after aggregation (out = num/denom), the
    segment-max subtraction is dropped (logits are tiny; exp is safe in fp32),
    and bias is folded into h (softmax weights sum to 1). ELU + final linear
    (z @ W2 + b2) run per destination tile, using a PE transpose.
"""

import os
import sys

import numpy as np

if "/opt/trn_rl_repo" not in sys.path:
    sys.path.insert(0, "/opt/trn_rl_repo")

N_NODES = 50000
N_EDGES = 800000
F_IN = 128
HEADS = 4
HIDDEN = 32
F_OUT = 64
NEG = 0.2
N_CORES = 8
P = 128
FE = F_IN + 2 * HEADS    # 136 phase-0 psum cols: h | alpha_s | alpha_d
FM = F_IN + HEADS        # 132 message cols: ex*h | ex
EB = 256                 # bf16 elements per htab row (512 B)
AS_OFF = 64              # f32 element offset of alpha_s within a row (byte 256)
LO_SPLIT = 32768         # int16 index limit for dma_gather
NODES_PER_CORE = N_NODES // N_CORES          # 6250
T_TILES = (NODES_PER_CORE + P - 1) // P      # 49
NPAD = T_TILES * 8 * P                       # 50176

# PLAIN_GATHER: use one [P,1]-indexed indirect DMA per edge slot instead of
# dma_gather (slower but uses only the scatter_add-congruent DMA form).
PLAIN_GATHER = True


def _prep(x, edge_index, W, a_src, a_dst):
    """CPU-side sharding: sort edges by destination, split by dst node range,
    sort each tile's edges by src and split at the int16 boundary, lay out
    wrapped int16 gather indices padded to a common capacity."""
    src = np.ascontiguousarray(np.asarray(edge_index[0]).astype(np.int64))
    dst = np.ascontiguousarray(np.asarray(edge_index[1]).astype(np.int64))

    A_s = np.zeros((F_IN, HEADS), np.float32)
    A_d = np.zeros((F_IN, HEADS), np.float32)
    for h in range(HEADS):
        A_s[h * HIDDEN:(h + 1) * HIDDEN, h] = a_src[h]
        A_d[h * HIDDEN:(h + 1) * HIDDEN, h] = a_dst[h]
    W_ext = np.concatenate([W, W @ A_s, W @ A_d], axis=1).astype(np.float32)

    # order edges by (core, tile, src) via a single composite sort
    core_of = dst // NODES_PER_CORE
    ltile_of = (dst - core_of * NODES_PER_CORE) // P
    group = core_of * T_TILES + ltile_of
    order = np.lexsort((src, group))
    src_s, dst_s, group_s = src[order], dst[order], group[order]
    lo_mask = src_s < LO_SPLIT

    NG = N_CORES * T_TILES
    gs = np.searchsorted(group_s, np.arange(NG))
    ge = np.searchsorted(group_s, np.arange(NG), side="right")
    # lo edges come first within each group (src-sorted)
    n_lo = np.array([np.count_nonzero(lo_mask[gs[g]:ge[g]]) for g in range(NG)])
    n_hi = (ge - gs) - n_lo
    K_LO = max(1, int(np.max((n_lo + P - 1) // P)))
    K_HI = max(1, int(np.max((n_hi + P - 1) // P)))
    K = K_LO + K_HI

    lo_idx = np.zeros((N_CORES, T_TILES, P, 8 * K_LO), np.int16)
    hi_idx = np.zeros((N_CORES, T_TILES, P, 8 * K_HI), np.int16)
    src32 = np.zeros((N_CORES, T_TILES, P, K), np.int32)
    d_local = np.full((N_CORES, T_TILES, P, K), -1.0, np.float32)
    dst_nodes = np.zeros((N_CORES, T_TILES, P, 1), np.int32)

    def wrap16(idx, n_slots):
        # index i lives at [i % 16, i // 16], replicated 8x over partitions
        full = np.zeros(n_slots * P, np.int16)
        full[:len(idx)] = idx
        return np.tile(full.reshape(n_slots * 8, 16).T, (8, 1))

    for c in range(N_CORES):
        for t in range(T_TILES):
            g = c * T_TILES + t
            s, e = gs[g], ge[g]
            nl = n_lo[g]
            base = c * NODES_PER_CORE + t * P
            lo_idx[c, t] = wrap16(src_s[s:s + nl].astype(np.int16), K_LO)
            hi_idx[c, t] = wrap16(
                (src_s[s + nl:e] - LO_SPLIT).astype(np.int16), K_HI)
            # edge i of the tile -> slot j, partition p
            i_lo = np.arange(nl)
            d_local[c, t, i_lo % P, i_lo // P] = (
                dst_s[s:s + nl] - base).astype(np.float32)
            src32[c, t, i_lo % P, i_lo // P] = src_s[s:s + nl]
            i_hi = np.arange(e - s - nl)
            d_local[c, t, i_hi % P, K_LO + i_hi // P] = (
                dst_s[s + nl:e] - base).astype(np.float32)
            src32[c, t, i_hi % P, K_LO + i_hi // P] = src_s[s + nl:e]
            nodes = base + np.arange(P)
            dst_nodes[c, t, :, 0] = np.minimum(nodes, N_NODES - 1)
    return W_ext, lo_idx, hi_idx, src32, d_local, dst_nodes, K_LO, K_HI


def _build_module(K_LO, K_HI, bias_nz, b2_nz, t_limit=None, p0_chunks=None,
                  skip=()):
    import concourse.bass as bass
    import concourse.mybir as mybir
    import concourse.tile as tile
    from concourse import bacc
    from concourse.masks import make_identity

    f32 = mybir.dt.float32
    bf16 = mybir.dt.bfloat16
    i16 = mybir.dt.int16
    i32 = mybir.dt.int32
    K = K_LO + K_HI

    nc = bacc.Bacc("TRN2", target_bir_lowering=False, debug=False,
                   num_devices=N_CORES)

    x_T = nc.dram_tensor("x_T", [P, NPAD], f32, kind="ExternalInput")
    W_ext_d = nc.dram_tensor("W_ext", [P, FE], f32, kind="ExternalInput")
    W2_d = nc.dram_tensor("W2", [P, F_OUT], f32, kind="ExternalInput")
    if PLAIN_GATHER:
        s32_d = nc.dram_tensor("src32", [T_TILES, P, K], i32,
                               kind="ExternalInput")
    else:
        lo_d = nc.dram_tensor("lo_idx", [T_TILES, P, 8 * K_LO], i16,
                              kind="ExternalInput")
        hi_d = nc.dram_tensor("hi_idx", [T_TILES, P, 8 * K_HI], i16,
                              kind="ExternalInput")
    dloc_d = nc.dram_tensor("d_local", [T_TILES, P, K], f32,
                            kind="ExternalInput")
    dstn_d = nc.dram_tensor("dst_nodes", [T_TILES, P, 1], i32,
                            kind="ExternalInput")
    if bias_nz:
        bias_d = nc.dram_tensor("bias_ext", [1, FE], f32, kind="ExternalInput")
    if b2_nz:
        b2_d = nc.dram_tensor("b2_row", [1, F_OUT], f32, kind="ExternalInput")
    y_d = nc.dram_tensor("y_out", [T_TILES * P, F_OUT], f32,
                         kind="ExternalOutput")
    htab = nc.dram_tensor("htab", [NPAD, EB], bf16, kind="Internal")
    ad_tab = nc.dram_tensor("ad_tab", [NPAD, HEADS], f32, kind="Internal")

    add = mybir.AluOpType.add
    mult = mybir.AluOpType.mult
    amax = mybir.AluOpType.max
    is_eq = mybir.AluOpType.is_equal
    Exp = mybir.ActivationFunctionType.Exp

    htab_f32 = htab.ap().bitcast(f32)  # [NPAD, 128]

    with tile.TileContext(nc) as tc:
        with tc.tile_pool(name="const", bufs=1) as constp:
            W_ext_sb = constp.tile([P, FE], f32)
            nc.sync.dma_start(W_ext_sb[:], W_ext_d.ap())
            W2_sb = constp.tile([P, F_OUT], f32)
            nc.sync.dma_start(W2_sb[:], W2_d.ap())
            iota_sb = constp.tile([P, P], f32)
            nc.gpsimd.iota(iota_sb[:], pattern=[[1, P]], base=0,
                           channel_multiplier=0,
                           allow_small_or_imprecise_dtypes=True)
            ident = constp.tile([P, P], f32)
            make_identity(nc, ident[:])
            if PLAIN_GATHER:
                s32_sb = constp.tile([P, T_TILES, K], i32)
                nc.sync.dma_start(s32_sb[:],
                                  s32_d.ap().rearrange("t p k -> p t k"))
            else:
                lo_sb = constp.tile([P, T_TILES, 8 * K_LO], i16)
                nc.sync.dma_start(lo_sb[:],
                                  lo_d.ap().rearrange("t p k -> p t k"))
                hi_sb = constp.tile([P, T_TILES, 8 * K_HI], i16)
                nc.sync.dma_start(hi_sb[:],
                                  hi_d.ap().rearrange("t p k -> p t k"))
            dloc_sb = constp.tile([P, T_TILES, K], f32)
            nc.sync.dma_start(dloc_sb[:],
                              dloc_d.ap().rearrange("t p k -> p t k"))
            dstn_sb = constp.tile([P, T_TILES], i32)
            nc.sync.dma_start(dstn_sb[:],
                              dstn_d.ap().rearrange("t p one -> p (t one)"))
            if bias_nz or b2_nz:
                ones_sb = constp.tile([1, P], f32)
                nc.vector.memset(ones_sb[:], 1.0)
            if bias_nz:
                bias_sb = constp.tile([1, FE], f32)
                nc.sync.dma_start(bias_sb[:], bias_d.ap())
            if b2_nz:
                b2_sb = constp.tile([1, F_OUT], f32)
                nc.sync.dma_start(b2_sb[:], b2_d.ap())

            # ---- phase 0: htab/ad_tab = x @ W_ext (+ bias on h cols) ----
            CH = 8  # node tiles per chunk
            with (
                tc.tile_pool(name="xt", bufs=3) as xtp,
                tc.tile_pool(name="hx", bufs=3) as hxp,
                tc.tile_pool(name="p0ps", bufs=4, space="PSUM") as p0ps,
            ):
                for c in range(p0_chunks or (NPAD // (CH * P))):
                    xt = xtp.tile([P, CH * P], f32)
                    nc.sync.dma_start(
                        xt[:], x_T.ap()[:, c * CH * P:(c + 1) * CH * P])
                    hxh = hxp.tile([P, CH * F_IN], bf16, tag="hxh")
                    # full f32 tail of each row (alpha_s + zero pad) so htab
                    # has no uninitialized bytes
                    TW = P - AS_OFF  # 64
                    hxa = hxp.tile([P, CH * TW], f32, tag="hxa")
                    nc.vector.memset(hxa[:], 0.0)
                    hxd = hxp.tile([P, CH * HEADS], f32, tag="hxd")
                    for j in range(CH):
                        ps = p0ps.tile([P, FE], f32)
                        nc.tensor.matmul(ps[:], lhsT=xt[:, j * P:(j + 1) * P],
                                         rhs=W_ext_sb[:], start=True,
                                         stop=not bias_nz)
                        if bias_nz:
                            nc.tensor.matmul(ps[:], lhsT=ones_sb[:],
                                             rhs=bias_sb[:], start=False,
                                             stop=True)
                        nc.vector.tensor_copy(
                            hxh[:, j * F_IN:(j + 1) * F_IN], ps[:, 0:F_IN])
                        nc.vector.tensor_copy(
                            hxa[:, j * TW:j * TW + HEADS],
                            ps[:, F_IN:F_IN + HEADS])
                        nc.vector.tensor_copy(
                            hxd[:, j * HEADS:(j + 1) * HEADS],
                            ps[:, F_IN + HEADS:FE])
                    rows = slice(c * CH * P, (c + 1) * CH * P)
                    nc.sync.dma_start(
                        htab.ap()[rows, 0:F_IN]
                        .rearrange("(t p) e -> p t e", p=P),
                        hxh[:].rearrange("p (t e) -> p t e", t=CH))
                    nc.sync.dma_start(
                        htab_f32[rows, AS_OFF:P]
                        .rearrange("(t p) e -> p t e", p=P),
                        hxa[:].rearrange("p (t e) -> p t e", t=CH))
                    nc.sync.dma_start(
                        ad_tab.ap()[rows, :]
                        .rearrange("(t p) e -> p t e", p=P),
                        hxd[:].rearrange("p (t e) -> p t e", t=CH))

            # ---- phase 1: per destination tile ----
            with (
                tc.tile_pool(name="g", bufs=2) as gp,
                tc.tile_pool(name="msgs", bufs=2) as mp,
                tc.tile_pool(name="S", bufs=2) as sp,
                tc.tile_pool(name="ST", bufs=2) as stp,
                tc.tile_pool(name="agg", bufs=2, space="PSUM") as aggp,
                tc.tile_pool(name="stps", bufs=2, space="PSUM") as stpsp,
                tc.tile_pool(name="adps", bufs=1, space="PSUM") as adpsp,
                tc.tile_pool(name="small", bufs=2) as smallp,
                tc.tile_pool(name="tr", bufs=2, space="PSUM") as trp,
                tc.tile_pool(name="yps", bufs=1, space="PSUM") as ypsp,
            ):
                for t in range(t_limit or T_TILES):
                    g = gp.tile([P, K * EB], bf16)
                    g3 = g[:].rearrange("p (k e) -> p k e", k=K)
                    if "gather" in skip:
                        nc.vector.memset(g[:], 0.5)
                    elif PLAIN_GATHER:
                        # one [P,1]-indexed row gather per edge slot
                        # (dest row 256 elems == table row; scatter_add form)
                        for j in range(K):
                            nc.gpsimd.indirect_dma_start(
                                out=g3[:, j, :], out_offset=None,
                                in_=htab.ap(),
                                in_offset=bass.IndirectOffsetOnAxis(
                                    ap=s32_sb[:, t, j:j + 1], axis=0))
                    else:
                        # dma_gather crashes above ~256 indices; issue
                        # 256-index chunks, each from a fresh packed idx tile
                        # (the exact configuration proven on HW)
                        CS = 2  # slots per chunk = 256 indices
                        for cs in range(0, K_LO, CS):
                            ns = min(CS, K_LO - cs)
                            ic = smallp.tile([P, 8 * CS], i16, tag="ic")
                            nc.vector.tensor_copy(
                                ic[:, 0:8 * ns],
                                lo_sb[:, t, 8 * cs:8 * (cs + ns)])
                            nc.gpsimd.dma_gather(
                                out_ap=g3[:, cs:cs + ns, :],
                                in_ap=htab.ap(),
                                idxs_ap=ic[:, 0:8 * ns],
                                num_idxs=ns * P, num_idxs_reg=ns * P,
                                elem_size=EB)
                        for cs in range(0, K_HI, CS):
                            ns = min(CS, K_HI - cs)
                            ic = smallp.tile([P, 8 * CS], i16, tag="ic2")
                            nc.vector.tensor_copy(
                                ic[:, 0:8 * ns],
                                hi_sb[:, t, 8 * cs:8 * (cs + ns)])
                            nc.gpsimd.dma_gather(
                                out_ap=g3[:, K_LO + cs:K_LO + cs + ns, :],
                                in_ap=htab.ap()[LO_SPLIT:, :],
                                idxs_ap=ic[:, 0:8 * ns],
                                num_idxs=ns * P, num_idxs_reg=ns * P,
                                elem_size=EB)
                    adt = smallp.tile([P, HEADS], f32, tag="adt")
                    if "adt" in skip:
                        nc.vector.memset(adt[:], 0.1)
                    else:
                        nc.gpsimd.indirect_dma_start(
                            out=adt[:], out_offset=None, in_=ad_tab.ap(),
                            in_offset=bass.IndirectOffsetOnAxis(
                                ap=dstn_sb[:, t:t + 1], axis=0))

                    # one-hot scatter matrix S[p=e, (k, d)]
                    S = sp.tile([P, K * P], f32)
                    nc.vector.tensor_tensor(
                        out=S[:].rearrange("p (k d) -> p k d", k=K),
                        in0=iota_sb[:].unsqueeze(1).to_broadcast([P, K, P]),
                        in1=dloc_sb[:, t, :].unsqueeze(2).to_broadcast(
                            [P, K, P]),
                        op=is_eq)
                    # S_T via PE transpose (DVE transpose is 32x32-block-local)
                    ST = stp.tile([P, K * P], f32)
                    if "st" in skip:
                        nc.vector.memset(ST[:], 0.0)
                    else:
                        for j in range(K):
                            stps = stpsp.tile([P, P], f32)
                            nc.tensor.transpose(out=stps[:],
                                                in_=S[:, j * P:(j + 1) * P],
                                                identity=ident[:])
                            nc.vector.tensor_copy(ST[:, j * P:(j + 1) * P],
                                                  stps[:])
                    # alpha_d per edge: [P, K*4] = S_T.T @ ad_tile per slot
                    adps = adpsp.tile([P, K * HEADS], f32)
                    if "admm" in skip:
                        nc.vector.memset(adps[:], 0.1)
                    else:
                        for j in range(K):
                            nc.tensor.matmul(
                                adps[:, j * HEADS:(j + 1) * HEADS],
                                lhsT=ST[:, j * P:(j + 1) * P], rhs=adt[:],
                                start=True, stop=True)

                    msgs = mp.tile([P, K * FM], f32)
                    m3 = msgs[:].rearrange("p (k f) -> p k f", k=K)
                    ex = m3[:, :, F_IN:FM]                       # [P, K, 4]
                    gf = g[:].bitcast(f32).rearrange("p (k e) -> p k e", k=K)
                    nc.vector.tensor_tensor(
                        out=ex, in0=gf[:, :, AS_OFF:AS_OFF + HEADS],
                        in1=adps[:].rearrange("p (k h) -> p k h", k=K),
                        op=add)
                    nc.vector.scalar_tensor_tensor(
                        out=ex, in0=ex, scalar=NEG, in1=ex,
                        op0=mult, op1=amax)
                    nc.scalar.activation(out=ex, in_=ex, func=Exp)
                    nc.vector.tensor_tensor(
                        out=m3[:, :, 0:F_IN].rearrange(
                            "p k (h f) -> p k h f", h=HEADS),
                        in0=g3[:, :, 0:F_IN].rearrange(
                            "p k (h f) -> p k h f", h=HEADS),
                        in1=ex.unsqueeze(3).to_broadcast(
                            [P, K, HEADS, HIDDEN]),
                        op=mult)

                    ps = aggp.tile([P, FM], f32)
                    for j in range(K):
                        nc.tensor.matmul(ps[:], lhsT=S[:, j * P:(j + 1) * P],
                                         rhs=msgs[:, j * FM:(j + 1) * FM],
                                         start=(j == 0), stop=(j == K - 1))

                    rec = smallp.tile([P, HEADS], f32, tag="rec")
                    nc.vector.tensor_scalar_add(out=rec[:],
                                                in0=ps[:, F_IN:FM],
                                                scalar1=1e-16)
                    nc.vector.reciprocal(rec[:], rec[:])
                    zn = smallp.tile([P, F_IN], f32, tag="zn")
                    nc.vector.tensor_tensor(
                        out=zn[:].rearrange("p (h f) -> p h f", h=HEADS),
                        in0=ps[:, 0:F_IN].rearrange("p (h f) -> p h f",
                                                    h=HEADS),
                        in1=rec[:].unsqueeze(2).to_broadcast(
                            [P, HEADS, HIDDEN]),
                        op=mult)
                    # ELU(z) = max(z, exp(min(z,0)) - 1)
                    tmp = smallp.tile([P, F_IN], f32, tag="tmp")
                    nc.vector.tensor_scalar_min(out=tmp[:], in0=zn[:],
                                                scalar1=0.0)
                    nc.scalar.activation(out=tmp[:], in_=tmp[:], func=Exp)
                    nc.vector.scalar_tensor_tensor(
                        out=zn[:], in0=tmp[:], scalar=-1.0, in1=zn[:],
                        op0=add, op1=amax)

                    pt = trp.tile([P, P], f32, tag="pt")
                    nc.tensor.transpose(out=pt[:], in_=zn[:],
                                        identity=ident[:])
                    znT = smallp.tile([P, P], f32, tag="znT")
                    nc.vector.tensor_copy(znT[:], pt[:])
                    yp = ypsp.tile([P, F_OUT], f32, tag="yp")
                    nc.tensor.matmul(yp[:], lhsT=znT[:], rhs=W2_sb[:],
                                     start=True, stop=not b2_nz)
                    if b2_nz:
                        nc.tensor.matmul(yp[:], lhsT=ones_sb[:], rhs=b2_sb[:],
                                         start=False, stop=True)
                    ysb = smallp.tile([P, F_OUT], f32, tag="ysb")
                    nc.vector.tensor_copy(ysb[:], yp[:])
                    nc.sync.dma_start(y_d.ap()[t * P:(t + 1) * P, :], ysb[:])

    nc.compile()
    return nc


_MODULE_CACHE = {}


def _get_module(K_LO, K_HI, bias_nz, b2_nz):
    key = (K_LO, K_HI, bias_nz, b2_nz, PLAIN_GATHER)
    if key not in _MODULE_CACHE:
        _MODULE_CACHE[key] = _build_module(K_LO, K_HI, bias_nz, b2_nz)
    return _MODULE_CACHE[key]


def _ensure_ntff_hook():
    """The axon NTFF profile hook lives in antenv.axon_hooks, which this
    image's antenv package lacks; shim it so trace=True works."""
    try:
        import antenv.axon_hooks  # noqa: F401
        return
    except ImportError:
        pass
    import types

    import antenv

    mod = types.ModuleType("antenv.axon_hooks")
    holder = {"h": None}
    mod.set_axon_ntff_profile_hook = lambda h: holder.__setitem__("h", h)
    mod.get_axon_ntff_profile_hook = lambda: holder["h"]
    try:
        from trn_agent_boot.trn_boot import _ntff_profile_via_ctypes
        holder["h"] = _ntff_profile_via_ctypes("/opt/axon/libaxon_pjrt.so")
    except Exception:
        pass
    sys.modules["antenv.axon_hooks"] = mod
    antenv.axon_hooks = mod


def kernel(x, edge_index, edge_weight, W, a_src, a_dst, bias, W2, b2,
           _trace=False):
    from concourse.bass_utils import run_bass_kernel_spmd

    if _trace:
        _ensure_ntff_hook()

    x = np.ascontiguousarray(np.asarray(x, np.float32))
    W = np.asarray(W, np.float32)
    a_src = np.asarray(a_src, np.float32)
    a_dst = np.asarray(a_dst, np.float32)
    bias = np.asarray(bias, np.float32)
    W2 = np.ascontiguousarray(np.asarray(W2, np.float32))
    b2 = np.asarray(b2, np.float32)

    W_ext, lo_idx, hi_idx, src32, d_local, dst_nodes, K_LO, K_HI = _prep(
        x, edge_index, W, a_src, a_dst)

    bias_nz = bool(np.any(bias))
    b2_nz = bool(np.any(b2))
    nc = _get_module(K_LO, K_HI, bias_nz, b2_nz)

    x_T = np.zeros((P, NPAD), np.float32)
    x_T[:, :N_NODES] = x.T

    in_maps = []
    for c in range(N_CORES):
        m = {
            "x_T": x_T,
            "W_ext": W_ext,
            "W2": W2,
            "d_local": np.ascontiguousarray(d_local[c]),
            "dst_nodes": np.ascontiguousarray(dst_nodes[c]),
        }
        if PLAIN_GATHER:
            m["src32"] = np.ascontiguousarray(src32[c])
        else:
            m["lo_idx"] = np.ascontiguousarray(lo_idx[c])
            m["hi_idx"] = np.ascontiguousarray(hi_idx[c])
        if bias_nz:
            be = np.zeros((1, FE), np.float32)
            be[0, :F_IN] = bias
            m["bias_ext"] = be
        if b2_nz:
            m["b2_row"] = b2.reshape(1, F_OUT)
        in_maps.append(m)

    res = run_bass_kernel_spmd(nc, in_maps, core_ids=list(range(N_CORES)),
                               trace=_trace)
    out = np.concatenate(
        [res.results[c]["y_out"][:NODES_PER_CORE] for c in range(N_CORES)],
        axis=0)
    if _trace:
        kernel.last_results = res
    return out



# revision 2
# speedup vs baseline: 1.0022x; 1.0022x over previous
"""GAT layer (nn_GAT_40037685133531) as a Trainium2 Bass kernel on 8 NeuronCores.

v4 strategy (graph/data parallel, no collectives):
  - Destination nodes sharded 8 ways (6250 per core); edges grouped by the
    128-node destination tile owning their dst, packed into K slots of 128.
  - Phase 0 (replicated, bf16): h_ext = x @ [W | W@A_s | W@A_d] -> htab
    [NPAD, 136] bf16 rows = [h bf16 x128 | alpha_s f32 x4] (272B); alpha_d
    accumulated in SBUF and written once to ad_tab [NPAD, 4] f32.
  - Phase 1 per tile: K x [P,1]-offset indirect DMAs gather the edge rows
    by src (measured: ~1.1us/op fixed issue cost on gpsimd is the kernel
    floor; multi-offset/dma_gather/ap_gather forms all measured slower).
    alpha_d for the tile's 128 dst nodes comes from one [P,1] gather on
    ad_tab and is broadcast to edges via bf16 PE transposes of the one-hot
    S (built by DVE is_equal vs an iota). ex = exp(leakyrelu(as+ad)),
    msgs = [ex*h | ex] bf16, psum += S_j.T @ msgs_j aggregates weighted
    sums + softmax denominators. Normalize, ELU, PE-transpose, z @ W2 in
    bf16; y accumulated in SBUF and written once.
"""

import os
import sys

import numpy as np

if "/opt/trn_rl_repo" not in sys.path:
    sys.path.insert(0, "/opt/trn_rl_repo")

N_NODES = 50000
N_EDGES = 800000
F_IN = 128
HEADS = 4
HIDDEN = 32
F_OUT = 64
NEG = 0.2
N_CORES = 8
P = 128
FE = F_IN + 2 * HEADS    # 136 phase-0 psum cols: h | alpha_s | alpha_d
FM = F_IN + HEADS        # 132 message cols: ex*h | ex
EB = FE                  # bf16 elements per htab row (272 B)
AS_OFF = 64              # f32 element offset of alpha_s within a row
NODES_PER_CORE = N_NODES // N_CORES          # 6250
T_TILES = (NODES_PER_CORE + P - 1) // P      # 49
NPAD = T_TILES * 8 * P                       # 50176
CH = 14                                      # phase-0 node tiles per chunk

def _prep(edge_index, W, a_src, a_dst):
    """CPU-side: extended weights; edges sorted by (core, tile, src) and
    packed into [P, K] slot layout per destination tile."""
    src = np.asarray(edge_index[0]).astype(np.int64)
    dst = np.asarray(edge_index[1]).astype(np.int64)

    A_s = np.zeros((F_IN, HEADS), np.float32)
    A_d = np.zeros((F_IN, HEADS), np.float32)
    for h in range(HEADS):
        A_s[h * HIDDEN:(h + 1) * HIDDEN, h] = a_src[h]
        A_d[h * HIDDEN:(h + 1) * HIDDEN, h] = a_dst[h]
    W_ext = np.concatenate([W, W @ A_s, W @ A_d], axis=1).astype(np.float32)

    core_of = dst // NODES_PER_CORE
    ltile_of = (dst - core_of * NODES_PER_CORE) // P
    group = core_of * T_TILES + ltile_of
    order = np.lexsort((src, group))
    src_s, dst_s, group_s = src[order], dst[order], group[order]

    NG = N_CORES * T_TILES
    gs = np.searchsorted(group_s, np.arange(NG))
    ge = np.searchsorted(group_s, np.arange(NG), side="right")
    cnt = (ge - gs).reshape(N_CORES, T_TILES)
    # Sort each core's tiles by edge count (desc): loop position t then only
    # needs k_list[t] = max over cores of the t-th largest slot count.
    perm = np.argsort(-cnt, axis=1)
    cnt_sorted = np.take_along_axis(cnt, perm, axis=1)
    k_list = [max(1, int(np.max((cnt_sorted[:, t] + P - 1) // P)))
              for t in range(T_TILES)]
    K = max(k_list)

    src32 = np.zeros((N_CORES, T_TILES, P, K), np.int32)
    d_local = np.full((N_CORES, T_TILES, P, K), -1.0, np.float32)
    dst_nodes = np.zeros((N_CORES, T_TILES, P, 1), np.int32)
    # dlocT[c, t, j*P + e] = d_local[c, t, e, j] (edge-slot-major, for STt)
    for c in range(N_CORES):
        for tp in range(T_TILES):
            t = perm[c, tp]
            g = c * T_TILES + t
            s, e = gs[g], ge[g]
            n = e - s
            base = c * NODES_PER_CORE + t * P
            i = np.arange(n)
            src32[c, tp, i % P, i // P] = src_s[s:e]
            d_local[c, tp, i % P, i // P] = (dst_s[s:e] - base)
            dst_nodes[c, tp, :, 0] = np.minimum(
                base + np.arange(P), N_NODES - 1)
    dlocT = np.ascontiguousarray(
        d_local.transpose(0, 1, 3, 2)).reshape(N_CORES, T_TILES, K * P)
    return W_ext, src32, d_local, dlocT, dst_nodes, perm, k_list


def _build_module(k_list, bias_nz, b2_nz):
    K = max(k_list)
    import concourse.bass as bass
    import concourse.mybir as mybir
    import concourse.tile as tile
    from concourse import bacc
    from concourse.masks import make_identity

    f32 = mybir.dt.float32
    bf16 = mybir.dt.bfloat16
    i32 = mybir.dt.int32

    nc = bacc.Bacc("TRN2", target_bir_lowering=False, debug=False,
                   num_devices=N_CORES)

    x_T = nc.dram_tensor("x_T", [P, NPAD], bf16, kind="ExternalInput")
    W_ext_d = nc.dram_tensor("W_ext", [P, FE], bf16, kind="ExternalInput")
    W2_d = nc.dram_tensor("W2", [P, F_OUT], bf16, kind="ExternalInput")
    s32_d = nc.dram_tensor("src32", [T_TILES, P, K], i32,
                           kind="ExternalInput")
    dstn_d = nc.dram_tensor("dst_nodes", [T_TILES, P, 1], i32,
                            kind="ExternalInput")
    dloc_d = nc.dram_tensor("d_local", [T_TILES, P, K], bf16,
                            kind="ExternalInput")
    dlocT_d = nc.dram_tensor("d_localT", [T_TILES, K * P], bf16,
                             kind="ExternalInput")
    if bias_nz:
        bias_d = nc.dram_tensor("bias_ext", [1, FE], bf16,
                                kind="ExternalInput")
    if b2_nz:
        b2_d = nc.dram_tensor("b2_row", [1, F_OUT], bf16,
                              kind="ExternalInput")
    y_d = nc.dram_tensor("y_out", [T_TILES * P, F_OUT], f32,
                         kind="ExternalOutput")
    htab = nc.dram_tensor("htab", [NPAD, EB], bf16, kind="Internal")
    ad_tab = nc.dram_tensor("ad_tab", [NPAD, HEADS], f32, kind="Internal")

    add = mybir.AluOpType.add
    mult = mybir.AluOpType.mult
    amax = mybir.AluOpType.max
    is_eq = mybir.AluOpType.is_equal
    Exp = mybir.ActivationFunctionType.Exp

    N_CHUNKS = NPAD // (CH * P)  # 28

    with tile.TileContext(nc) as tc:
        with tc.tile_pool(name="const", bufs=1) as constp:
            W_ext_sb = constp.tile([P, FE], bf16)
            nc.sync.dma_start(W_ext_sb[:], W_ext_d.ap())
            W2_sb = constp.tile([P, F_OUT], bf16)
            nc.sync.dma_start(W2_sb[:], W2_d.ap())
            iota_f = constp.tile([P, P], f32)
            nc.gpsimd.iota(iota_f[:], pattern=[[1, P]], base=0,
                           channel_multiplier=0,
                           allow_small_or_imprecise_dtypes=True)
            iota_sb = constp.tile([P, P], bf16)
            nc.vector.tensor_copy(iota_sb[:], iota_f[:])
            iotap_f = constp.tile([P, 1], f32)
            nc.gpsimd.iota(iotap_f[:], pattern=[[0, 1]], base=0,
                           channel_multiplier=1,
                           allow_small_or_imprecise_dtypes=True)
            iotap = constp.tile([P, 1], bf16)
            nc.vector.tensor_copy(iotap[:], iotap_f[:])
            ident_f = constp.tile([P, P], f32)
            make_identity(nc, ident_f[:])
            ident = constp.tile([P, P], bf16)
            nc.vector.tensor_copy(ident[:], ident_f[:])
            ones1 = constp.tile([1, P], bf16)
            nc.vector.memset(ones1[:], 1.0)
            s32_sb = constp.tile([P, T_TILES, K], i32)
            nc.sync.dma_start(s32_sb[:],
                              s32_d.ap().rearrange("t p k -> p t k"))
            dstn_sb = constp.tile([P, T_TILES], i32)
            nc.sync.dma_start(dstn_sb[:],
                              dstn_d.ap().rearrange("t p one -> p (t one)"))
            dloc_sb = constp.tile([P, T_TILES, K], bf16)
            nc.sync.dma_start(dloc_sb[:],
                              dloc_d.ap().rearrange("t p k -> p t k"))
            if bias_nz or b2_nz:
                ones_sb = constp.tile([1, P], bf16)
                nc.vector.memset(ones_sb[:], 1.0)
            if bias_nz:
                bias_sb = constp.tile([1, FE], bf16)
                nc.sync.dma_start(bias_sb[:], bias_d.ap())
            if b2_nz:
                b2_sb = constp.tile([1, F_OUT], bf16)
                nc.sync.dma_start(b2_sb[:], b2_d.ap())
            ad_acc = constp.tile([P, NPAD // P, HEADS], f32)
            y_acc = constp.tile([P, T_TILES, F_OUT], f32)

            # ---- phase 0: htab = [x@W_ext | as]; ad_acc = ad ----
            # 3 node-tiles share one PSUM bank so the PSUM->SBUF copies
            # amortize the DVE per-op overhead.
            with (
                tc.tile_pool(name="xt", bufs=3) as xtp,
                tc.tile_pool(name="hx", bufs=3) as hxp,
                tc.tile_pool(name="p0ps", bufs=4, space="PSUM") as p0ps,
            ):
                for c in range(N_CHUNKS):
                    xt = xtp.tile([P, CH * P], bf16)
                    nc.scalar.dma_start(
                        xt[:], x_T.ap()[:, c * CH * P:(c + 1) * CH * P])
                    hrow = hxp.tile([P, CH, EB], bf16, tag="hrow")
                    hrow_f32 = hrow[:].bitcast(f32)
                    for j0 in range(0, CH, 3):
                        nj = min(3, CH - j0)
                        ps = p0ps.tile([P, 3, FE], f32)
                        for j in range(j0, j0 + nj):
                            nc.tensor.matmul(
                                ps[:, j - j0, :],
                                lhsT=xt[:, j * P:(j + 1) * P],
                                rhs=W_ext_sb[:], start=True,
                                stop=not bias_nz)
                            if bias_nz:
                                nc.tensor.matmul(ps[:, j - j0, :],
                                                 lhsT=ones_sb[:],
                                                 rhs=bias_sb[:], start=False,
                                                 stop=True)
                        nc.vector.tensor_copy(
                            hrow[:, j0:j0 + nj, 0:F_IN],
                            ps[:, 0:nj, 0:F_IN])
                        nc.vector.tensor_copy(
                            hrow_f32[:, j0:j0 + nj, AS_OFF:AS_OFF + HEADS],
                            ps[:, 0:nj, F_IN:F_IN + HEADS])
                        nc.vector.tensor_copy(
                            ad_acc[:, c * CH + j0:c * CH + j0 + nj, :],
                            ps[:, 0:nj, F_IN + HEADS:FE])
                    rows = slice(c * CH * P, (c + 1) * CH * P)
                    nc.sync.dma_start(
                        htab.ap()[rows, :].rearrange("(t p) e -> p t e", p=P),
                        hrow[:])
                    if (c + 1) % 7 == 0:  # quarters: after chunks 6,13,20,27
                        q = slice((c - 6) * CH * P, (c + 1) * CH * P)
                        nc.sync.dma_start(
                            ad_tab.ap()[q, :].rearrange(
                                "(t p) e -> p t e", p=P),
                            ad_acc[:, (c - 6) * CH:(c + 1) * CH, :])

            # ---- phase 1: per destination tile ----
            BC = 512  # bcast-matmul chunk (one PSUM bank of f32)
            with (
                tc.tile_pool(name="g", bufs=6) as gp,
                tc.tile_pool(name="msgs", bufs=3) as mp,
                tc.tile_pool(name="S", bufs=2) as sp,
                tc.tile_pool(name="STt", bufs=2) as stp,
                tc.tile_pool(name="dlT", bufs=2) as dlp,
                tc.tile_pool(name="agg", bufs=2, space="PSUM") as aggp,
                tc.tile_pool(name="bcps", bufs=2, space="PSUM") as bcpsp,
                tc.tile_pool(name="adps", bufs=2, space="PSUM") as adpsp,
                tc.tile_pool(name="small", bufs=4) as smallp,
                tc.tile_pool(name="tr", bufs=1, space="PSUM") as trp,
                tc.tile_pool(name="yps", bufs=1, space="PSUM") as ypsp,
            ):
                for t in range(T_TILES):
                    Kt = k_list[t]
                    adt = smallp.tile([P, HEADS], f32, tag="adt")
                    nc.gpsimd.indirect_dma_start(
                        out=adt[:], out_offset=None, in_=ad_tab.ap(),
                        in_offset=bass.IndirectOffsetOnAxis(
                            ap=dstn_sb[:, t:t + 1], axis=0))
                    g = gp.tile([P, Kt, EB], bf16, tag="g")
                    for j in range(Kt):
                        nc.gpsimd.indirect_dma_start(
                            out=g[:, j, :], out_offset=None,
                            in_=htab.ap(),
                            in_offset=bass.IndirectOffsetOnAxis(
                                ap=s32_sb[:, t, j:j + 1], axis=0))
                    adtb = smallp.tile([P, HEADS], bf16, tag="adtb")
                    nc.vector.tensor_copy(adtb[:], adt[:])

                    # one-hot scatter matrix S[p=e, (k, d)] in bf16
                    S = sp.tile([P, Kt, P], bf16, tag="S")
                    nc.vector.tensor_tensor(
                        out=S[:],
                        in0=iota_sb[:].unsqueeze(1).to_broadcast([P, Kt, P]),
                        in1=dloc_sb[:, t, 0:Kt].unsqueeze(2).to_broadcast(
                            [P, Kt, P]),
                        op=is_eq)
                    # STt[d, (j e)] = (d == dloc[e, j]) built directly:
                    # dlocT row broadcast across partitions via PE, then one
                    # is_equal against the partition-index iota.
                    dlT_row = dlp.tile([1, Kt * P], bf16, tag="dlr")
                    nc.sync.dma_start(dlT_row[:],
                                      dlocT_d.ap()[t:t + 1, 0:Kt * P])
                    dlT = dlp.tile([P, Kt * P], bf16, tag="dlT")
                    for q0 in range(0, Kt * P, BC):
                        qn = min(BC, Kt * P - q0)
                        bps = bcpsp.tile([P, BC], f32)
                        nc.tensor.matmul(bps[:, 0:qn], lhsT=ones1[:],
                                         rhs=dlT_row[:, q0:q0 + qn],
                                         start=True, stop=True)
                        nc.vector.tensor_copy(dlT[:, q0:q0 + qn],
                                              bps[:, 0:qn])
                    STt = stp.tile([P, Kt * P], bf16, tag="STt")
                    nc.vector.tensor_tensor(
                        out=STt[:],
                        in0=iotap[:].to_broadcast([P, Kt * P]),
                        in1=dlT[:], op=is_eq)
                    # alpha_d per edge: adps[:, j*4:(j+1)*4] = STt_j.T @ adtb
                    adps = adpsp.tile([P, Kt * HEADS], f32, tag="adps")
                    for j in range(Kt):
                        nc.tensor.matmul(
                            adps[:, j * HEADS:(j + 1) * HEADS],
                            lhsT=STt[:, j * P:(j + 1) * P], rhs=adtb[:],
                            start=True, stop=True)

                    # ex = exp(leakyrelu(alpha_s + alpha_d)) per edge
                    gf = g[:].bitcast(f32)   # [P, Kt, 68]
                    ex = smallp.tile([P, Kt, HEADS], f32, tag="ex")
                    nc.vector.tensor_tensor(
                        out=ex[:], in0=gf[:, :, AS_OFF:AS_OFF + HEADS],
                        in1=adps[:].rearrange("p (k h) -> p k h", k=Kt),
                        op=add)
                    nc.vector.scalar_tensor_tensor(
                        out=ex[:], in0=ex[:], scalar=NEG, in1=ex[:],
                        op0=mult, op1=amax)
                    exb = smallp.tile([P, Kt, HEADS], bf16, tag="exb")
                    nc.scalar.activation(out=exb[:], in_=ex[:], func=Exp)

                    msgs = mp.tile([P, Kt, FM], bf16, tag="msgs")
                    nc.vector.tensor_tensor(
                        out=msgs[:, :, 0:F_IN].rearrange(
                            "p k (h f) -> p k h f", h=HEADS),
                        in0=g[:, :, 0:F_IN].rearrange(
                            "p k (h f) -> p k h f", h=HEADS),
                        in1=exb[:].unsqueeze(3).to_broadcast(
                            [P, Kt, HEADS, HIDDEN]),
                        op=mult)
                    nc.vector.tensor_copy(msgs[:, :, F_IN:FM], exb[:])

                    ps = aggp.tile([P, FM], f32)
                    for j in range(Kt):
                        nc.tensor.matmul(ps[:], lhsT=S[:, j, :],
                                         rhs=msgs[:, j, :],
                                         start=(j == 0), stop=(j == K - 1))

                    rec = smallp.tile([P, HEADS], f32, tag="rec")
                    nc.vector.tensor_scalar_add(out=rec[:],
                                                in0=ps[:, F_IN:FM],
                                                scalar1=1e-16)
                    nc.vector.reciprocal(rec[:], rec[:])
                    zn = smallp.tile([P, F_IN], f32, tag="zn")
                    nc.vector.tensor_tensor(
                        out=zn[:].rearrange("p (h f) -> p h f", h=HEADS),
                        in0=ps[:, 0:F_IN].rearrange("p (h f) -> p h f",
                                                    h=HEADS),
                        in1=rec[:].unsqueeze(2).to_broadcast(
                            [P, HEADS, HIDDEN]),
                        op=mult)
                    # ELU(z) = max(z, exp(min(z,0)) - 1)
                    tmp = smallp.tile([P, F_IN], f32, tag="tmp")
                    nc.vector.tensor_scalar_min(out=tmp[:], in0=zn[:],
                                                scalar1=0.0)
                    nc.scalar.activation(out=tmp[:], in_=tmp[:], func=Exp)
                    znb = smallp.tile([P, F_IN], bf16, tag="znb")
                    nc.vector.scalar_tensor_tensor(
                        out=znb[:], in0=tmp[:], scalar=-1.0, in1=zn[:],
                        op0=add, op1=amax)

                    pt = trp.tile([P, P], bf16, tag="pt")
                    nc.tensor.transpose(out=pt[:], in_=znb[:],
                                        identity=ident[:])
                    znT = smallp.tile([P, P], bf16, tag="znT")
                    nc.vector.tensor_copy(znT[:], pt[:])
                    yp = ypsp.tile([P, F_OUT], f32, tag="yp")
                    nc.tensor.matmul(yp[:], lhsT=znT[:], rhs=W2_sb[:],
                                     start=True, stop=not b2_nz)
                    if b2_nz:
                        nc.tensor.matmul(yp[:], lhsT=ones_sb[:], rhs=b2_sb[:],
                                         start=False, stop=True)
                    nc.vector.tensor_copy(y_acc[:, t, :], yp[:])
            nc.sync.dma_start(
                y_d.ap().rearrange("(t p) f -> p t f", p=P), y_acc[:])

    nc.compile()
    return nc


_MODULE_CACHE = {}


def _get_module(k_list, bias_nz, b2_nz):
    key = (tuple(k_list), bias_nz, b2_nz)
    if key not in _MODULE_CACHE:
        _MODULE_CACHE[key] = _build_module(k_list, bias_nz, b2_nz)
    return _MODULE_CACHE[key]


def _ensure_ntff_hook():
    """The axon NTFF profile hook lives in antenv.axon_hooks, which this
    image's antenv package lacks; shim it so trace=True works."""
    try:
        import antenv.axon_hooks  # noqa: F401
        return
    except ImportError:
        pass
    import types

    import antenv

    mod = types.ModuleType("antenv.axon_hooks")
    holder = {"h": None}
    mod.set_axon_ntff_profile_hook = lambda h: holder.__setitem__("h", h)
    mod.get_axon_ntff_profile_hook = lambda: holder["h"]
    try:
        from trn_agent_boot.trn_boot import _ntff_profile_via_ctypes
        holder["h"] = _ntff_profile_via_ctypes("/opt/axon/libaxon_pjrt.so")
    except Exception:
        pass
    sys.modules["antenv.axon_hooks"] = mod
    antenv.axon_hooks = mod


def kernel(x, edge_index, edge_weight, W, a_src, a_dst, bias, W2, b2,
           _trace=False):
    import ml_dtypes
    from concourse.bass_utils import run_bass_kernel_spmd

    bf = ml_dtypes.bfloat16
    if _trace:
        _ensure_ntff_hook()

    x = np.asarray(x, np.float32)
    W = np.asarray(W, np.float32)
    a_src = np.asarray(a_src, np.float32)
    a_dst = np.asarray(a_dst, np.float32)
    bias = np.asarray(bias, np.float32)
    W2 = np.asarray(W2, np.float32)
    b2 = np.asarray(b2, np.float32)

    W_ext, src32, d_local, dlocT, dst_nodes, perm, k_list = _prep(
        edge_index, W, a_src, a_dst)

    bias_nz = bool(np.any(bias))
    b2_nz = bool(np.any(b2))
    nc = _get_module(k_list, bias_nz, b2_nz)

    x_T = np.zeros((P, NPAD), bf)
    x_T[:, :N_NODES] = x.T.astype(bf)

    in_maps = []
    for c in range(N_CORES):
        m = {
            "x_T": x_T,
            "W_ext": W_ext.astype(bf),
            "W2": W2.astype(bf),
            "src32": np.ascontiguousarray(src32[c]),
            "dst_nodes": np.ascontiguousarray(dst_nodes[c]),
            "d_local": np.ascontiguousarray(d_local[c].astype(bf)),
            "d_localT": np.ascontiguousarray(dlocT[c].astype(bf)),
        }
        if bias_nz:
            be = np.zeros((1, FE), np.float32)
            be[0, :F_IN] = bias
            m["bias_ext"] = be.astype(bf)
        if b2_nz:
            m["b2_row"] = b2.reshape(1, F_OUT).astype(bf)
        in_maps.append(m)

    res = run_bass_kernel_spmd(nc, in_maps, core_ids=list(range(N_CORES)),
                               trace=_trace)
    parts = []
    for c in range(N_CORES):
        y = res.results[c]["y_out"].reshape(T_TILES, P, F_OUT)
        inv = np.argsort(perm[c])
        parts.append(y[inv].reshape(T_TILES * P, F_OUT)[:NODES_PER_CORE])
    out = np.concatenate(parts, axis=0)
    if _trace:
        kernel.last_results = res
    return out


# revision 3
# speedup vs baseline: 1.0248x; 1.0226x over previous
"""GAT layer (nn_GAT_40037685133531) as a Trainium2 Bass kernel on 8 NeuronCores.

v4 strategy (graph/data parallel, no collectives):
  - Destination nodes sharded 8 ways (6250 per core); edges grouped by the
    128-node destination tile owning their dst, packed into K slots of 128.
  - Phase 0 (replicated, bf16): h_ext = x @ [W | W@A_s | W@A_d] -> htab
    [NPAD, 136] bf16 rows = [h bf16 x128 | alpha_s f32 x4] (272B); alpha_d
    accumulated in SBUF and written once to ad_tab [NPAD, 4] f32.
  - Phase 1 per tile: K x [P,1]-offset indirect DMAs gather the edge rows
    by src (measured: ~1.1us/op fixed issue cost on gpsimd is the kernel
    floor; multi-offset/dma_gather/ap_gather forms all measured slower).
    alpha_d for the tile's 128 dst nodes comes from one [P,1] gather on
    ad_tab and is broadcast to edges via bf16 PE transposes of the one-hot
    S (built by DVE is_equal vs an iota). ex = exp(leakyrelu(as+ad)),
    msgs = [ex*h | ex] bf16, psum += S_j.T @ msgs_j aggregates weighted
    sums + softmax denominators. Normalize, ELU, PE-transpose, z @ W2 in
    bf16; y accumulated in SBUF and written once.
"""

import os
import sys

import numpy as np

if "/opt/trn_rl_repo" not in sys.path:
    sys.path.insert(0, "/opt/trn_rl_repo")

N_NODES = 50000
N_EDGES = 800000
F_IN = 128
HEADS = 4
HIDDEN = 32
F_OUT = 64
NEG = 0.2
N_CORES = 8
P = 128
FE = F_IN + 2 * HEADS    # 136 phase-0 psum cols: h | alpha_s | alpha_d
FM = F_IN + HEADS        # 132 message cols: ex*h | ex
EB = FE                  # bf16 elements per htab row (272 B)
AS_OFF = 64              # f32 element offset of alpha_s within a row
NODES_PER_CORE = N_NODES // N_CORES          # 6250
T_TILES = (NODES_PER_CORE + P - 1) // P      # 49
NPAD = T_TILES * 8 * P                       # 50176
CH = 14                                      # phase-0 node tiles per chunk

def _prep(edge_index, W, a_src, a_dst):
    """CPU-side: extended weights; edges sorted by (core, tile, src) and
    packed into [P, K] slot layout per destination tile."""
    src = np.asarray(edge_index[0]).astype(np.int64)
    dst = np.asarray(edge_index[1]).astype(np.int64)

    A_s = np.zeros((F_IN, HEADS), np.float32)
    A_d = np.zeros((F_IN, HEADS), np.float32)
    for h in range(HEADS):
        A_s[h * HIDDEN:(h + 1) * HIDDEN, h] = a_src[h]
        A_d[h * HIDDEN:(h + 1) * HIDDEN, h] = a_dst[h]
    W_ext = np.concatenate([W, W @ A_s, W @ A_d], axis=1).astype(np.float32)

    core_of = dst // NODES_PER_CORE
    ltile_of = (dst - core_of * NODES_PER_CORE) // P
    group = core_of * T_TILES + ltile_of
    order = np.lexsort((src, group))
    src_s, dst_s, group_s = src[order], dst[order], group[order]

    NG = N_CORES * T_TILES
    gs = np.searchsorted(group_s, np.arange(NG))
    ge = np.searchsorted(group_s, np.arange(NG), side="right")
    cnt = (ge - gs).reshape(N_CORES, T_TILES)
    # Sort each core's tiles by edge count (desc): loop position t then only
    # needs k_list[t] = max over cores of the t-th largest slot count.
    perm = np.argsort(-cnt, axis=1)
    cnt_sorted = np.take_along_axis(cnt, perm, axis=1)
    k_list = [max(1, int(np.max((cnt_sorted[:, t] + P - 1) // P)))
              for t in range(T_TILES)]
    K = max(k_list)

    src32 = np.zeros((N_CORES, T_TILES, P, K), np.int32)
    d_local = np.full((N_CORES, T_TILES, P, K), -1.0, np.float32)
    dst_nodes = np.zeros((N_CORES, T_TILES, P, 1), np.int32)
    # dlocT[c, t, j*P + e] = d_local[c, t, e, j] (edge-slot-major, for STt)
    for c in range(N_CORES):
        for tp in range(T_TILES):
            t = perm[c, tp]
            g = c * T_TILES + t
            s, e = gs[g], ge[g]
            n = e - s
            base = c * NODES_PER_CORE + t * P
            i = np.arange(n)
            src32[c, tp, i % P, i // P] = src_s[s:e]
            d_local[c, tp, i % P, i // P] = (dst_s[s:e] - base)
            dst_nodes[c, tp, :, 0] = np.minimum(
                base + np.arange(P), N_NODES - 1)
    dlocT = np.ascontiguousarray(
        d_local.transpose(0, 1, 3, 2)).reshape(N_CORES, T_TILES, K * P)
    return W_ext, src32, d_local, dlocT, dst_nodes, perm, k_list


def _build_module(k_list, bias_nz, b2_nz):
    K = max(k_list)
    import concourse.bass as bass
    import concourse.mybir as mybir
    import concourse.tile as tile
    from concourse import bacc
    from concourse.masks import make_identity

    f32 = mybir.dt.float32
    bf16 = mybir.dt.bfloat16
    i32 = mybir.dt.int32

    nc = bacc.Bacc("TRN2", target_bir_lowering=False, debug=False,
                   num_devices=N_CORES, dynamic_dma_scratch_size=32768)

    x_T = nc.dram_tensor("x_T", [P, NPAD], bf16, kind="ExternalInput")
    W_ext_d = nc.dram_tensor("W_ext", [P, FE], bf16, kind="ExternalInput")
    W2_d = nc.dram_tensor("W2", [P, F_OUT], bf16, kind="ExternalInput")
    s32_d = nc.dram_tensor("src32", [T_TILES, P, K], i32,
                           kind="ExternalInput")
    dstn_d = nc.dram_tensor("dst_nodes", [T_TILES, P, 1], i32,
                            kind="ExternalInput")
    dloc_d = nc.dram_tensor("d_local", [T_TILES, P, K], bf16,
                            kind="ExternalInput")
    dlocT_d = nc.dram_tensor("d_localT", [T_TILES, K * P], bf16,
                             kind="ExternalInput")
    if bias_nz:
        bias_d = nc.dram_tensor("bias_ext", [1, FE], bf16,
                                kind="ExternalInput")
    if b2_nz:
        b2_d = nc.dram_tensor("b2_row", [1, F_OUT], bf16,
                              kind="ExternalInput")
    y_d = nc.dram_tensor("y_out", [T_TILES * P, F_OUT], f32,
                         kind="ExternalOutput")
    htab = nc.dram_tensor("htab", [NPAD, EB], bf16, kind="Internal")
    ad_tab = nc.dram_tensor("ad_tab", [NPAD, HEADS], f32, kind="Internal")

    add = mybir.AluOpType.add
    mult = mybir.AluOpType.mult
    amax = mybir.AluOpType.max
    is_eq = mybir.AluOpType.is_equal
    Exp = mybir.ActivationFunctionType.Exp

    N_CHUNKS = NPAD // (CH * P)  # 28

    with tile.TileContext(nc) as tc:
        with tc.tile_pool(name="const", bufs=1) as constp:
            W_ext_sb = constp.tile([P, FE], bf16)
            nc.sync.dma_start(W_ext_sb[:], W_ext_d.ap())
            W2_sb = constp.tile([P, F_OUT], bf16)
            nc.sync.dma_start(W2_sb[:], W2_d.ap())
            iota_f = constp.tile([P, P], f32)
            nc.gpsimd.iota(iota_f[:], pattern=[[1, P]], base=0,
                           channel_multiplier=0,
                           allow_small_or_imprecise_dtypes=True)
            iota_sb = constp.tile([P, P], bf16)
            nc.vector.tensor_copy(iota_sb[:], iota_f[:])
            iotap_f = constp.tile([P, 1], f32)
            nc.gpsimd.iota(iotap_f[:], pattern=[[0, 1]], base=0,
                           channel_multiplier=1,
                           allow_small_or_imprecise_dtypes=True)
            iotap = constp.tile([P, 1], bf16)
            nc.vector.tensor_copy(iotap[:], iotap_f[:])
            ident_f = constp.tile([P, P], f32)
            make_identity(nc, ident_f[:])
            ident = constp.tile([P, P], bf16)
            nc.vector.tensor_copy(ident[:], ident_f[:])
            ones1 = constp.tile([1, P], bf16)
            nc.vector.memset(ones1[:], 1.0)
            s32_sb = constp.tile([P, T_TILES, K], i32)
            nc.sync.dma_start(s32_sb[:],
                              s32_d.ap().rearrange("t p k -> p t k"))
            dstn_sb = constp.tile([P, T_TILES], i32)
            nc.sync.dma_start(dstn_sb[:],
                              dstn_d.ap().rearrange("t p one -> p (t one)"))
            dloc_sb = constp.tile([P, T_TILES, K], bf16)
            nc.sync.dma_start(dloc_sb[:],
                              dloc_d.ap().rearrange("t p k -> p t k"))
            if bias_nz or b2_nz:
                ones_sb = constp.tile([1, P], bf16)
                nc.vector.memset(ones_sb[:], 1.0)
            if bias_nz:
                bias_sb = constp.tile([1, FE], bf16)
                nc.sync.dma_start(bias_sb[:], bias_d.ap())
            if b2_nz:
                b2_sb = constp.tile([1, F_OUT], bf16)
                nc.sync.dma_start(b2_sb[:], b2_d.ap())
            ad_acc = constp.tile([P, NPAD // P, HEADS], f32)
            y_acc = constp.tile([P, T_TILES, F_OUT], f32)

            # ---- phase 0: htab = [x@W_ext | as]; ad_acc = ad ----
            # 3 node-tiles share one PSUM bank so the PSUM->SBUF copies
            # amortize the DVE per-op overhead.
            with (
                tc.tile_pool(name="xt", bufs=3) as xtp,
                tc.tile_pool(name="hx", bufs=3) as hxp,
                tc.tile_pool(name="p0ps", bufs=4, space="PSUM") as p0ps,
            ):
                for c in range(N_CHUNKS):
                    xt = xtp.tile([P, CH * P], bf16)
                    nc.scalar.dma_start(
                        xt[:], x_T.ap()[:, c * CH * P:(c + 1) * CH * P])
                    hrow = hxp.tile([P, CH, EB], bf16, tag="hrow")
                    hrow_f32 = hrow[:].bitcast(f32)
                    for j0 in range(0, CH, 3):
                        nj = min(3, CH - j0)
                        ps = p0ps.tile([P, 3, FE], f32)
                        for j in range(j0, j0 + nj):
                            nc.tensor.matmul(
                                ps[:, j - j0, :],
                                lhsT=xt[:, j * P:(j + 1) * P],
                                rhs=W_ext_sb[:], start=True,
                                stop=not bias_nz)
                            if bias_nz:
                                nc.tensor.matmul(ps[:, j - j0, :],
                                                 lhsT=ones_sb[:],
                                                 rhs=bias_sb[:], start=False,
                                                 stop=True)
                        nc.vector.tensor_copy(
                            hrow[:, j0:j0 + nj, 0:F_IN],
                            ps[:, 0:nj, 0:F_IN])
                        nc.vector.tensor_copy(
                            hrow_f32[:, j0:j0 + nj, AS_OFF:AS_OFF + HEADS],
                            ps[:, 0:nj, F_IN:F_IN + HEADS])
                        nc.vector.tensor_copy(
                            ad_acc[:, c * CH + j0:c * CH + j0 + nj, :],
                            ps[:, 0:nj, F_IN + HEADS:FE])
                    rows = slice(c * CH * P, (c + 1) * CH * P)
                    nc.sync.dma_start(
                        htab.ap()[rows, :].rearrange("(t p) e -> p t e", p=P),
                        hrow[:])
                    if (c + 1) % 7 == 0:  # quarters: after chunks 6,13,20,27
                        q = slice((c - 6) * CH * P, (c + 1) * CH * P)
                        nc.sync.dma_start(
                            ad_tab.ap()[q, :].rearrange(
                                "(t p) e -> p t e", p=P),
                            ad_acc[:, (c - 6) * CH:(c + 1) * CH, :])

            # ---- phase 1: per destination tile ----
            BC = 512  # bcast-matmul chunk (one PSUM bank of f32)
            with (
                tc.tile_pool(name="g", bufs=6) as gp,
                tc.tile_pool(name="msgs", bufs=3) as mp,
                tc.tile_pool(name="S", bufs=2) as sp,
                tc.tile_pool(name="STt", bufs=2) as stp,
                tc.tile_pool(name="dlT", bufs=2) as dlp,
                tc.tile_pool(name="agg", bufs=2, space="PSUM") as aggp,
                tc.tile_pool(name="bcps", bufs=2, space="PSUM") as bcpsp,
                tc.tile_pool(name="adps", bufs=2, space="PSUM") as adpsp,
                tc.tile_pool(name="small", bufs=4) as smallp,
                tc.tile_pool(name="tr", bufs=1, space="PSUM") as trp,
                tc.tile_pool(name="yps", bufs=1, space="PSUM") as ypsp,
            ):
                for t in range(T_TILES):
                    Kt = k_list[t]
                    adt = smallp.tile([P, HEADS], f32, tag="adt")
                    nc.gpsimd.indirect_dma_start(
                        out=adt[:], out_offset=None, in_=ad_tab.ap(),
                        in_offset=bass.IndirectOffsetOnAxis(
                            ap=dstn_sb[:, t:t + 1], axis=0))
                    g = gp.tile([P, Kt, EB], bf16, tag="g")
                    for j in range(Kt):
                        nc.gpsimd.indirect_dma_start(
                            out=g[:, j, :], out_offset=None,
                            in_=htab.ap(),
                            in_offset=bass.IndirectOffsetOnAxis(
                                ap=s32_sb[:, t, j:j + 1], axis=0))
                    adtb = smallp.tile([P, HEADS], bf16, tag="adtb")
                    nc.vector.tensor_copy(adtb[:], adt[:])

                    # one-hot scatter matrix S[p=e, (k, d)] in bf16
                    S = sp.tile([P, Kt, P], bf16, tag="S")
                    nc.vector.tensor_tensor(
                        out=S[:],
                        in0=iota_sb[:].unsqueeze(1).to_broadcast([P, Kt, P]),
                        in1=dloc_sb[:, t, 0:Kt].unsqueeze(2).to_broadcast(
                            [P, Kt, P]),
                        op=is_eq)
                    # STt[d, (j e)] = (d == dloc[e, j]) built directly:
                    # dlocT row broadcast across partitions via PE, then one
                    # is_equal against the partition-index iota.
                    dlT_row = dlp.tile([1, Kt * P], bf16, tag="dlr")
                    nc.sync.dma_start(dlT_row[:],
                                      dlocT_d.ap()[t:t + 1, 0:Kt * P])
                    dlT = dlp.tile([P, Kt * P], bf16, tag="dlT")
                    for q0 in range(0, Kt * P, BC):
                        qn = min(BC, Kt * P - q0)
                        bps = bcpsp.tile([P, BC], f32)
                        nc.tensor.matmul(bps[:, 0:qn], lhsT=ones1[:],
                                         rhs=dlT_row[:, q0:q0 + qn],
                                         start=True, stop=True)
                        nc.vector.tensor_copy(dlT[:, q0:q0 + qn],
                                              bps[:, 0:qn])
                    STt = stp.tile([P, Kt * P], bf16, tag="STt")
                    nc.vector.tensor_tensor(
                        out=STt[:],
                        in0=iotap[:].to_broadcast([P, Kt * P]),
                        in1=dlT[:], op=is_eq)
                    # alpha_d per edge: adps[:, j*4:(j+1)*4] = STt_j.T @ adtb
                    adps = adpsp.tile([P, Kt * HEADS], f32, tag="adps")
                    for j in range(Kt):
                        nc.tensor.matmul(
                            adps[:, j * HEADS:(j + 1) * HEADS],
                            lhsT=STt[:, j * P:(j + 1) * P], rhs=adtb[:],
                            start=True, stop=True)

                    # ex = exp(leakyrelu(alpha_s + alpha_d)) per edge
                    gf = g[:].bitcast(f32)   # [P, Kt, 68]
                    ex = smallp.tile([P, Kt, HEADS], f32, tag="ex")
                    nc.vector.tensor_tensor(
                        out=ex[:], in0=gf[:, :, AS_OFF:AS_OFF + HEADS],
                        in1=adps[:].rearrange("p (k h) -> p k h", k=Kt),
                        op=add)
                    nc.vector.scalar_tensor_tensor(
                        out=ex[:], in0=ex[:], scalar=NEG, in1=ex[:],
                        op0=mult, op1=amax)
                    exb = smallp.tile([P, Kt, HEADS], bf16, tag="exb")
                    nc.scalar.activation(out=exb[:], in_=ex[:], func=Exp)

                    msgs = mp.tile([P, Kt, FM], bf16, tag="msgs")
                    nc.vector.tensor_tensor(
                        out=msgs[:, :, 0:F_IN].rearrange(
                            "p k (h f) -> p k h f", h=HEADS),
                        in0=g[:, :, 0:F_IN].rearrange(
                            "p k (h f) -> p k h f", h=HEADS),
                        in1=exb[:].unsqueeze(3).to_broadcast(
                            [P, Kt, HEADS, HIDDEN]),
                        op=mult)
                    nc.vector.tensor_copy(msgs[:, :, F_IN:FM], exb[:])

                    ps = aggp.tile([P, FM], f32)
                    for j in range(Kt):
                        nc.tensor.matmul(ps[:], lhsT=S[:, j, :],
                                         rhs=msgs[:, j, :],
                                         start=(j == 0), stop=(j == K - 1))

                    rec = smallp.tile([P, HEADS], f32, tag="rec")
                    nc.vector.tensor_scalar_add(out=rec[:],
                                                in0=ps[:, F_IN:FM],
                                                scalar1=1e-16)
                    nc.vector.reciprocal(rec[:], rec[:])
                    zn = smallp.tile([P, F_IN], f32, tag="zn")
                    nc.vector.tensor_tensor(
                        out=zn[:].rearrange("p (h f) -> p h f", h=HEADS),
                        in0=ps[:, 0:F_IN].rearrange("p (h f) -> p h f",
                                                    h=HEADS),
                        in1=rec[:].unsqueeze(2).to_broadcast(
                            [P, HEADS, HIDDEN]),
                        op=mult)
                    # ELU(z) = max(z, exp(min(z,0)) - 1)
                    tmp = smallp.tile([P, F_IN], f32, tag="tmp")
                    nc.vector.tensor_scalar_min(out=tmp[:], in0=zn[:],
                                                scalar1=0.0)
                    nc.scalar.activation(out=tmp[:], in_=tmp[:], func=Exp)
                    znb = smallp.tile([P, F_IN], bf16, tag="znb")
                    nc.vector.scalar_tensor_tensor(
                        out=znb[:], in0=tmp[:], scalar=-1.0, in1=zn[:],
                        op0=add, op1=amax)

                    pt = trp.tile([P, P], bf16, tag="pt")
                    nc.tensor.transpose(out=pt[:], in_=znb[:],
                                        identity=ident[:])
                    znT = smallp.tile([P, P], bf16, tag="znT")
                    nc.vector.tensor_copy(znT[:], pt[:])
                    yp = ypsp.tile([P, F_OUT], f32, tag="yp")
                    nc.tensor.matmul(yp[:], lhsT=znT[:], rhs=W2_sb[:],
                                     start=True, stop=not b2_nz)
                    if b2_nz:
                        nc.tensor.matmul(yp[:], lhsT=ones_sb[:], rhs=b2_sb[:],
                                         start=False, stop=True)
                    nc.vector.tensor_copy(y_acc[:, t, :], yp[:])
            nc.sync.dma_start(
                y_d.ap().rearrange("(t p) f -> p t f", p=P), y_acc[:])

    nc.compile()
    return nc


_MODULE_CACHE = {}


def _get_module(k_list, bias_nz, b2_nz):
    key = (tuple(k_list), bias_nz, b2_nz)
    if key not in _MODULE_CACHE:
        _MODULE_CACHE[key] = _build_module(k_list, bias_nz, b2_nz)
    return _MODULE_CACHE[key]


def _ensure_ntff_hook():
    """The axon NTFF profile hook lives in antenv.axon_hooks, which this
    image's antenv package lacks; shim it so trace=True works."""
    try:
        import antenv.axon_hooks  # noqa: F401
        return
    except ImportError:
        pass
    import types

    import antenv

    mod = types.ModuleType("antenv.axon_hooks")
    holder = {"h": None}
    mod.set_axon_ntff_profile_hook = lambda h: holder.__setitem__("h", h)
    mod.get_axon_ntff_profile_hook = lambda: holder["h"]
    try:
        from trn_agent_boot.trn_boot import _ntff_profile_via_ctypes
        holder["h"] = _ntff_profile_via_ctypes("/opt/axon/libaxon_pjrt.so")
    except Exception:
        pass
    sys.modules["antenv.axon_hooks"] = mod
    antenv.axon_hooks = mod


def kernel(x, edge_index, edge_weight, W, a_src, a_dst, bias, W2, b2,
           _trace=False):
    import ml_dtypes
    from concourse.bass_utils import run_bass_kernel_spmd

    bf = ml_dtypes.bfloat16
    if _trace:
        _ensure_ntff_hook()

    x = np.asarray(x, np.float32)
    W = np.asarray(W, np.float32)
    a_src = np.asarray(a_src, np.float32)
    a_dst = np.asarray(a_dst, np.float32)
    bias = np.asarray(bias, np.float32)
    W2 = np.asarray(W2, np.float32)
    b2 = np.asarray(b2, np.float32)

    W_ext, src32, d_local, dlocT, dst_nodes, perm, k_list = _prep(
        edge_index, W, a_src, a_dst)

    bias_nz = bool(np.any(bias))
    b2_nz = bool(np.any(b2))
    nc = _get_module(k_list, bias_nz, b2_nz)

    x_T = np.zeros((P, NPAD), bf)
    x_T[:, :N_NODES] = x.T.astype(bf)

    in_maps = []
    for c in range(N_CORES):
        m = {
            "x_T": x_T,
            "W_ext": W_ext.astype(bf),
            "W2": W2.astype(bf),
            "src32": np.ascontiguousarray(src32[c]),
            "dst_nodes": np.ascontiguousarray(dst_nodes[c]),
            "d_local": np.ascontiguousarray(d_local[c].astype(bf)),
            "d_localT": np.ascontiguousarray(dlocT[c].astype(bf)),
        }
        if bias_nz:
            be = np.zeros((1, FE), np.float32)
            be[0, :F_IN] = bias
            m["bias_ext"] = be.astype(bf)
        if b2_nz:
            m["b2_row"] = b2.reshape(1, F_OUT).astype(bf)
        in_maps.append(m)

    res = run_bass_kernel_spmd(nc, in_maps, core_ids=list(range(N_CORES)),
                               trace=_trace)
    parts = []
    for c in range(N_CORES):
        y = res.results[c]["y_out"].reshape(T_TILES, P, F_OUT)
        inv = np.argsort(perm[c])
        parts.append(y[inv].reshape(T_TILES * P, F_OUT)[:NODES_PER_CORE])
    out = np.concatenate(parts, axis=0)
    if _trace:
        kernel.last_results = res
    return out
